# revision 5
# baseline (speedup 1.0000x reference)
"""Trainium2 Bass kernel for a CPPN-style dense MLP forward pass.

Network (per pixel): 11 -> [32 x 23 tanh layers] -> 3 sigmoid.
  h = tanh(x @ W1.T); 22x: h = tanh(h @ Whid[l].T); out = sigmoid(h @ Wout.T)

Full inputs:  x [4194304, 11] f32, W1 [32, 11], Whid [22, 32, 32], Wout [3, 32]
Full output:  [4194304, 3] f32

Strategy: pure data parallel over 8 NeuronCores (pixels split 8 ways,
weights replicated).  Per core the kernel is ScalarE(tanh)-throughput
bound, so the layout keeps ACT ~100% busy on large [128, 2048]
activations while the PE runs the 32x32 matmuls 16-at-a-time via
tile_position packing (all 16 32x32 sub-arrays concurrently).

Layout per core: pixels processed in "supertiles" of 16 tiles x 512
pixels = 8192 pixels.  Activations live feature-major: tile (a,b) holds
[32 features, 512 pixels] at SBUF partitions [32u:32u+32], free offset
512*v, where (u,v)=(a,b) on even layers and (b,a) on odd layers.  Each
layer = 16 concurrent matmuls at tile_position (32u, 32v) writing one
[128, 2048] PSUM half (4 banks), then one big ACT tanh PSUM->SBUF.
Two supertile streams are interleaved (PSUM ping-pong) so the PE fills
one PSUM half while ACT drains the other.

I/O avoids small-packet DMA death: x is loaded pixel-major with 44B
contiguous chunks and block-transposed to feature-major on the (idle)
VectorE via its 32x32 STREAM_TRANSPOSE; the sigmoid output is
block-transposed back so the store scatters 12B/pixel chunks with a
32-row outer dim (spreads across all 16 DMA engines).  All DMAs are
issued from SyncE - DMA issue occupies the issuing engine's
instruction stream and must stay off the ACT critical path.

Matmuls are full fp32 (2-pass LOW/HIGH on the PE).  The 24-layer tanh
chain is chaotic (Lyapunov growth ~700x): fp32 implementations already
differ from each other by ~2e-4 L2 on the final output, and reduced
matmul precision (float32r, ~12 mantissa bits, would be 2x faster and
single-pass) amplifies to ~0.14 L2 - unusable.  Measured: ~3.20 ms on
hardware, vs a 2.95 ms ScalarE floor (1536 ACTIVATEs x (2048+222)cyc
@ 1.2 GHz); PE/DVE/DMA are all hidden under the tanh stream.
"""

import os
import sys

if "/opt/trn_rl_repo" not in sys.path:
    sys.path.insert(0, "/opt/trn_rl_repo")

import numpy as np

N_CORES = 8
N_PIX = 4194304
P_CORE = N_PIX // N_CORES      # 524288 pixels per core
D_IN = 11
D_H = 32
N_LAYERS = 24                  # 1 input + 22 hidden + 1 output
F = 512                        # pixels per tile (one PSUM bank of fp32)
ST_PIX = 16 * F                # 8192 pixels per supertile
N_ST = P_CORE // ST_PIX        # 64 supertiles per core
N_PAIRS = N_ST // 2            # 32 interleaved supertile pairs

_BUILD_CACHE = {}


def _build(n_pairs, f32r_from):
    """Build + bass-compile the per-core program. Returns the Bacc object."""
    import concourse.bass as bass  # noqa: F401
    import concourse.tile as tile
    from concourse import bacc, mybir
    from contextlib import ExitStack

    f32 = mybir.dt.float32
    f32r = mybir.dt.float32r
    Tanh = mybir.ActivationFunctionType.Tanh
    Sigmoid = mybir.ActivationFunctionType.Sigmoid

    nc = bacc.Bacc(
        "TRN2", target_bir_lowering=False, debug=False, num_devices=N_CORES
    )
    x_ap = nc.dram_tensor("x", [P_CORE, D_IN], f32, kind="ExternalInput").ap()
    w_ap = nc.dram_tensor("w", [128, N_LAYERS * 32], f32, kind="ExternalInput").ap()
    wbd_ap = nc.dram_tensor("wbd", [128, 23 * 128], f32, kind="ExternalInput").ap()
    o_ap = nc.dram_tensor("o", [P_CORE, 3], f32, kind="ExternalOutput").ap()

    with tile.TileContext(nc) as tc, ExitStack() as ctx:
        wp = ctx.enter_context(tc.tile_pool(name="wp", bufs=1))
        xrp = ctx.enter_context(tc.tile_pool(name="xrp", bufs=4))
        xp = ctx.enter_context(tc.tile_pool(name="xp", bufs=4))
        hp = ctx.enter_context(tc.tile_pool(name="hp", bufs=4))
        sp = ctx.enter_context(tc.tile_pool(name="sp", bufs=6))
        pp = ctx.enter_context(tc.tile_pool(name="pp", bufs=2, space="PSUM"))

        Wf = wp.tile([128, N_LAYERS * 32], f32)
        nc.sync.dma_start(Wf[:], w_ap[:])
        if f32r_from < N_LAYERS:
            # Trailing layers run as full-array [128,128] block-diagonal f32r
            # matmuls (f32r only supports column-group 0, so no 16-way
            # packing); the explicit scalar copy is the required f32r
            # rounding producer.  Chaotic error growth caps how many layers
            # tolerate ~12-bit mantissa: f32r from layer 12 measures ~5e-3
            # final L2 (gate is 2e-2); keep f32r_from even so the fp32
            # 16-way grid swaps cancel by the store.
            Wbf = wp.tile([128, 23 * 128], f32)
            nc.sync.dma_start(Wbf[:], wbd_ap[:])
            Wbr = wp.tile([128, 23 * 128], f32r)
            nc.scalar.copy(Wbr[:], Wbf[:])

        def load_x(s, eng):
            # Stage 1: pixel-major load, 44B contiguous chunks per pixel row,
            # laid out so that 32x32 block-transpose yields feature-major
            # tiles: XR[32u+p, 32c+f] = x[s*8192 + u*2048 + 32c + p, f].
            XR = xrp.tile([128, 2048], f32)
            for u in range(4):
                p0 = s * ST_PIX + u * 2048
                eng.dma_start(
                    XR[32 * u : 32 * u + 32, :].rearrange(
                        "p (c f) -> p c f", c=64, f=32
                    )[:, :, 0:D_IN],
                    x_ap[p0 : p0 + 2048, :].rearrange("(c p) f -> p c f", c=64, p=32),
                )
            # Stage 2: DVE 32x32 block transpose -> X[32u+f, 32c+p].
            X = xp.tile([128, 2048], f32)
            nc.vector.transpose(X[:], XR[:])
            return X


        def layer(H, k):
            """One layer for one supertile: matmuls + one ACT."""
            Kd = D_IN if k == 0 else 32
            last = k == N_LAYERS - 1
            P_ = pp.tile([128, 2048], f32)
            if k >= f32r_from:
                # 4 block-diagonal full-array f32r matmuls, layout-
                # preserving: tile (a,g) stays at [32g, 512a].  Block k-1
                # of Wbr holds layer k's weights (incl. the padded output
                # layer at block 22).
                for a in range(4):
                    nc.tensor.matmul(
                        P_[:, 512 * a : 512 * a + 512],
                        lhsT=Wbr[:, 128 * (k - 1) : 128 * k],
                        rhs=H[:, 512 * a : 512 * a + 512],
                        start=True,
                        stop=True,
                        tile_position=(0, 0),
                    )
            else:
                # fp32 16-way tile-position packing; iterate so consecutive
                # matmuls land on different PE row groups (LDWEIGHTS only
                # pulls ahead of in-flight MMs when row_grp differs).
                ab = [(a, b) for b in range(4) for a in range(4)]
                if k % 2 == 1:
                    ab = [(a, b) for a in range(4) for b in range(4)]
                for a, b in ab:
                        u, v = (a, b) if k % 2 == 0 else (b, a)
                        nc.tensor.matmul(
                            P_[32 * v : 32 * v + 32, 512 * u : 512 * u + 512],
                            lhsT=Wf[32 * u : 32 * u + Kd, 32 * k : 32 * k + 32],
                            rhs=H[32 * u : 32 * u + Kd, 512 * v : 512 * v + 512],
                            start=True,
                            stop=True,
                            tile_position=(32 * u, 32 * v),
                        )
            if last:
                Hn = sp.tile([128, 2048], f32)
                nc.scalar.activation(Hn[:], P_[:], Sigmoid)
            else:
                # Output dtype f32r iff the next layer's matmul is f32r.
                h_dt = f32r if k + 1 >= f32r_from else f32
                Hn = hp.tile([128, 2048], h_dt)
                nc.scalar.activation(Hn[:], P_[:], Tanh)
            return Hn

        def store_out(s, S, eng):
            # Block-transpose back to pixel-major so the scatter uses 12B
            # chunks with a 32-row outer dim (spreads across all DMA engines):
            # SR[32a+p, 32c+f] = S[32a+f, 32c+p] = out feature f of pixel
            # s*8192 + a*2048 + 32c + p.
            SR = sp.tile([128, 2048], f32)
            nc.vector.transpose(SR[:], S[:])
            for a in range(4):
                p0 = s * ST_PIX + a * 2048
                eng.dma_start(
                    o_ap[p0 : p0 + 2048, :].rearrange("(c p) f -> p c f", c=64, p=32),
                    SR[32 * a : 32 * a + 32, :].rearrange(
                        "p (c f) -> p c f", c=64, f=32
                    )[:, :, 0:3],
                )

        # Software-pipelined input staging: issue pair p+1's loads (DMA +
        # DVE transpose) before pair p's store transposes so the DVE queue
        # doesn't head-of-line block the next pair's first tanh behind
        # stores that wait on this pair's sigmoid.
        XA, XB = load_x(0, nc.sync), load_x(1, nc.sync)
        for pair in range(n_pairs):
            sA, sB = 2 * pair, 2 * pair + 1
            HA, HB = XA, XB
            if pair + 1 < n_pairs:
                XA, XB = load_x(sA + 2, nc.sync), load_x(sB + 2, nc.sync)
            for k in range(N_LAYERS):
                HA = layer(HA, k)
                HB = layer(HB, k)
            store_out(sA, HA, nc.sync)
            store_out(sB, HB, nc.sync)

    nc.compile()
    return nc


def _get_program(n_pairs, f32r_from):
    key = (n_pairs, f32r_from)
    if key not in _BUILD_CACHE:
        _BUILD_CACHE[key] = _build(n_pairs, f32r_from)
    return _BUILD_CACHE[key]


def _pack_weights(W1, Whid, Wout):
    """[128, 24*32]: per partition-group u, column block l*32 holds W_l.T."""
    WT = np.zeros((N_LAYERS, 32, 32), np.float32)
    WT[0, :D_IN, :] = np.asarray(W1, np.float32).T
    WT[1:23] = np.transpose(np.asarray(Whid, np.float32), (0, 2, 1))
    WT[23, :, :3] = np.asarray(Wout, np.float32).T
    Wh = np.zeros((128, N_LAYERS * 32), np.float32)
    blocks = WT.transpose(1, 0, 2).reshape(32, N_LAYERS * 32)
    for u in range(4):
        Wh[32 * u : 32 * u + 32, :] = blocks
    Wbd = np.zeros((128, 23, 128), np.float32)
    for g in range(4):
        Wbd[32 * g : 32 * g + 32, :, 32 * g : 32 * g + 32] = WT[1:24].transpose(
            1, 0, 2
        )
    return Wh, Wbd.reshape(128, 23 * 128)


def _run(x, W1, Whid, Wout, trace=False, n_pairs=None, **spmd_kwargs):
    from concourse.bass_utils import run_bass_kernel_spmd

    if n_pairs is None:
        n_pairs = int(os.environ.get("BASSK_PAIRS", N_PAIRS))
    f32r_from = int(os.environ.get("BASSK_F32R_FROM", 12))
    nc = _get_program(n_pairs, f32r_from)

    x = np.ascontiguousarray(np.asarray(x, np.float32))
    assert x.shape == (N_PIX, D_IN), x.shape
    Wh, Wbd = _pack_weights(W1, Whid, Wout)

    in_maps = [
        {"x": x[i * P_CORE : (i + 1) * P_CORE], "w": Wh, "wbd": Wbd}
        for i in range(N_CORES)
    ]
    res = run_bass_kernel_spmd(
        nc, in_maps, list(range(N_CORES)), trace=trace, **spmd_kwargs
    )
    out = np.concatenate([res.results[i]["o"] for i in range(N_CORES)], axis=0)
    return out, res


def kernel(x, W1, Whid, Wout):
    out, _ = _run(x, W1, Whid, Wout)
    return out



# revision 14
# speedup vs baseline: 1.0375x; 1.0375x over previous
"""Trainium2 Bass kernel for a CPPN-style dense MLP forward pass.

Network (per pixel): 11 -> [32 x 23 tanh layers] -> 3 sigmoid.
  h = tanh(x @ W1.T); 22x: h = tanh(h @ Whid[l].T); out = sigmoid(h @ Wout.T)

Full inputs:  x [4194304, 11] f32, W1 [32, 11], Whid [22, 32, 32], Wout [3, 32]
Full output:  [4194304, 3] f32

Strategy: pure data parallel over 8 NeuronCores (pixels split 8 ways,
weights replicated).  Per core the kernel is ScalarE(tanh)-throughput
bound, so the layout keeps ACT ~100% busy on large [128, 2048]
activations while the PE runs the 32x32 matmuls 16-at-a-time via
tile_position packing (all 16 32x32 sub-arrays concurrently).

Layout per core: pixels processed in "supertiles" of 16 tiles x 512
pixels = 8192 pixels.  Activations live feature-major: tile (a,b) holds
[32 features, 512 pixels] at SBUF partitions [32u:32u+32], free offset
512*v, where (u,v)=(a,b) on even layers and (b,a) on odd layers.  Each
layer = 16 concurrent matmuls at tile_position (32u, 32v) writing one
[128, 2048] PSUM half (4 banks), then one big ACT tanh PSUM->SBUF.
Two supertile streams are interleaved (PSUM ping-pong) so the PE fills
one PSUM half while ACT drains the other.

I/O avoids small-packet DMA death: x is loaded pixel-major with 44B
contiguous chunks and block-transposed to feature-major on the (idle)
VectorE via its 32x32 STREAM_TRANSPOSE; the sigmoid output is
block-transposed back so the store scatters 12B/pixel chunks with a
32-row outer dim (spreads across all 16 DMA engines).  All DMAs are
issued from SyncE - DMA issue occupies the issuing engine's
instruction stream and must stay off the ACT critical path.

Matmuls are full fp32 (2-pass LOW/HIGH on the PE).  The 24-layer tanh
chain is chaotic (Lyapunov growth ~700x): fp32 implementations already
differ from each other by ~2e-4 L2 on the final output, and reduced
matmul precision (float32r, ~12 mantissa bits, would be 2x faster and
single-pass) amplifies to ~0.14 L2 - unusable.  Measured: ~3.20 ms on
hardware, vs a 2.95 ms ScalarE floor (1536 ACTIVATEs x (2048+222)cyc
@ 1.2 GHz); PE/DVE/DMA are all hidden under the tanh stream.
"""

import os
import sys

if "/opt/trn_rl_repo" not in sys.path:
    sys.path.insert(0, "/opt/trn_rl_repo")

import numpy as np

N_CORES = 8
N_PIX = 4194304
P_CORE = N_PIX // N_CORES      # 524288 pixels per core
D_IN = 11
D_H = 32
N_LAYERS = 24                  # 1 input + 22 hidden + 1 output
F = 512                        # pixels per tile (one PSUM bank of fp32)
ST_PIX = 16 * F                # 8192 pixels per supertile
N_ST = P_CORE // ST_PIX        # 64 supertiles per core
N_PAIRS = N_ST // 2            # 32 interleaved supertile pairs

_BUILD_CACHE = {}

# Rational tanh for the DVE tail: tanh(x) ~= x*P(x^2)/Q(x^2) on [-5, 5]
# (clamped), P/Q cubic with unit constant terms (fit maxerr 6e-8; the
# clamp's saturation error 9.1e-5 dominates).  1/Q via the stock
# RECIPROCAL_APPROX_FAST op (~51 ULP).
A_CLAMP = 5.0
RAT_P = (0.12547021940727285, 0.0025078209274810507, 5.175999166403778e-06)
RAT_Q = (0.458802999833475, 0.02210980598237151, 0.00016889688396419473)

_DVE_OPS = None


def _register_dve_ops():
    """Register the 3 custom DVE ops (append-only; idempotent)."""
    global _DVE_OPS
    if _DVE_OPS is not None:
        return _DVE_OPS
    import numpy as np_
    import concourse.dve_ops as dve_ops
    from concourse.dve_ops import DveOp
    from concourse.dve_spec import (
        C0, C1, C2, One, Spec, Src0, Src1, lower, sq, _has_src1,
    )
    from concourse.dve_uop import DveOpSpec

    existing = {op.name: op for op in dve_ops.OPS}

    def mk(name, spec):
        if name in existing:
            return existing[name]
        op = DveOp(name, spec, subdim=False, uops_sha={})
        dve_ops.OPS.append(op)
        dve_ops._SUB_OPCODE_FOR_NAME[name] = (
            dve_ops._CUSTOM_DVE_ROW_BASE + len(dve_ops.OPS) - 1
        )
        dve_ops.CUSTOM_DVE_SPECS[name] = spec
        compiled = DveOpSpec(
            name=name,
            opcode=dve_ops.get_dve_sub_opcode(name),
            uops=lower(spec, ver="v3"),
            rd1_en=_has_src1(spec),
        )
        op.uops_sha["v3"] = compiled.sha("v3")
        return op

    U1 = sq(Src0)
    num = mk("TANH_NUM_ANT2", Spec(
        body=(((C0 * U1 + C1) * U1 + C2) * U1 + One) * Src0,
        reference=lambda in0, in1, s0, s1, imm2: (
            (((s0 * (in0 * in0) + s1) * (in0 * in0) + imm2) * (in0 * in0)
             + np_.float32(1.0)) * in0
        ).astype(np_.float32),
    ))
    U2 = sq(Src0)
    den = mk("TANH_DEN_ANT2", Spec(
        body=((C0 * U2 + C1) * U2 + C2) * U2 + One,
        reference=lambda in0, in1, s0, s1, imm2: (
            ((s0 * (in0 * in0) + s1) * (in0 * in0) + imm2) * (in0 * in0)
            + np_.float32(1.0)
        ).astype(np_.float32),
    ))
    ma = mk("MUL_ADDC_ANT2", Spec(
        body=(Src0 * Src1) * C0 + C1,
        reference=lambda in0, in1, s0, s1, imm2: (
            in0 * in1 * s0 + s1
        ).astype(np_.float32),
    ))
    _DVE_OPS = (num, den, ma)
    return _DVE_OPS


def _build(n_pairs, f32r_from, tail_from):
    """Build + bass-compile the per-core program. Returns the Bacc object."""
    import concourse.bass as bass  # noqa: F401
    import concourse.tile as tile
    from concourse import bacc, mybir
    import concourse.dve_ops as dve_ops
    from contextlib import ExitStack

    f32 = mybir.dt.float32
    f32r = mybir.dt.float32r
    Tanh = mybir.ActivationFunctionType.Tanh
    Sigmoid = mybir.ActivationFunctionType.Sigmoid
    tail_on = tail_from < N_LAYERS
    if tail_on:
        OP_NUM, OP_DEN, OP_MA = _register_dve_ops()
        assert tail_from > f32r_from, "tail layers must be in the f32r range"

    nc = bacc.Bacc(
        "TRN2", target_bir_lowering=False, debug=False, num_devices=N_CORES
    )
    x_ap = nc.dram_tensor("x", [P_CORE, D_IN], f32, kind="ExternalInput").ap()
    w_ap = nc.dram_tensor("w", [128, N_LAYERS * 32], f32, kind="ExternalInput").ap()
    wbd_ap = nc.dram_tensor("wbd", [128, 23 * 128], f32, kind="ExternalInput").ap()
    o_ap = nc.dram_tensor("o", [P_CORE, 3], f32, kind="ExternalOutput").ap()

    with tile.TileContext(nc) as tc, ExitStack() as ctx:
        wp = ctx.enter_context(tc.tile_pool(name="wp", bufs=1))
        xrp = ctx.enter_context(tc.tile_pool(name="xrp", bufs=2 if tail_on else 4))
        xp = ctx.enter_context(tc.tile_pool(name="xp", bufs=2 if tail_on else 4))
        hp = ctx.enter_context(tc.tile_pool(name="hp", bufs=3 if tail_on else 4))
        sp = ctx.enter_context(tc.tile_pool(name="sp", bufs=2 if tail_on else 4))
        if tail_on:
            tp = ctx.enter_context(tc.tile_pool(name="tp", bufs=2))
            thp = ctx.enter_context(tc.tile_pool(name="thp", bufs=2))
        pp = ctx.enter_context(tc.tile_pool(name="pp", bufs=2, space="PSUM"))

        Wf = wp.tile([128, N_LAYERS * 32], f32)
        nc.sync.dma_start(Wf[:], w_ap[:])
        if f32r_from < N_LAYERS:
            # Trailing layers run as full-array [128,128] block-diagonal f32r
            # matmuls (f32r only supports column-group 0, so no 16-way
            # packing); the explicit scalar copy is the required f32r
            # rounding producer.  Chaotic error growth caps how many layers
            # tolerate ~12-bit mantissa: f32r from layer 12 measures ~5e-3
            # final L2 (gate is 2e-2); keep f32r_from even so the fp32
            # 16-way grid swaps cancel by the store.
            Wbf = wp.tile([128, 23 * 128], f32)
            nc.sync.dma_start(Wbf[:], wbd_ap[:])
            Wbr = wp.tile([128, 23 * 128], f32r)
            nc.scalar.copy(Wbr[:], Wbf[:])

        def load_x(s, eng):
            # Stage 1: pixel-major load, 44B contiguous chunks per pixel row,
            # laid out so that 32x32 block-transpose yields feature-major
            # tiles: XR[32u+p, 32c+f] = x[s*8192 + u*2048 + 32c + p, f].
            XR = xrp.tile([128, 2048], f32)
            for u in range(4):
                p0 = s * ST_PIX + u * 2048
                eng.dma_start(
                    XR[32 * u : 32 * u + 32, :].rearrange(
                        "p (c f) -> p c f", c=64, f=32
                    )[:, :, 0:D_IN],
                    x_ap[p0 : p0 + 2048, :].rearrange("(c p) f -> p c f", c=64, p=32),
                )
            # Stage 2: DVE 32x32 block transpose -> X[32u+f, 32c+p].
            X = xp.tile([128, 2048], f32)
            nc.vector.transpose(X[:], XR[:])
            return X


        def layer(H, k):
            """One layer for one supertile: matmuls + one ACT."""
            Kd = D_IN if k == 0 else 32
            last = k == N_LAYERS - 1
            P_ = pp.tile([128, 2048], f32)
            if k >= f32r_from:
                # 4 block-diagonal full-array f32r matmuls, layout-
                # preserving: tile (a,g) stays at [32g, 512a].  Block k-1
                # of Wbr holds layer k's weights (incl. the padded output
                # layer at block 22).
                for a in range(4):
                    nc.tensor.matmul(
                        P_[:, 512 * a : 512 * a + 512],
                        lhsT=Wbr[:, 128 * (k - 1) : 128 * k],
                        rhs=H[:, 512 * a : 512 * a + 512],
                        start=True,
                        stop=True,
                        tile_position=(0, 0),
                    )
            else:
                # fp32 16-way tile-position packing; iterate so consecutive
                # matmuls land on different PE row groups (LDWEIGHTS only
                # pulls ahead of in-flight MMs when row_grp differs).
                ab = [(a, b) for b in range(4) for a in range(4)]
                if k % 2 == 1:
                    ab = [(a, b) for a in range(4) for b in range(4)]
                for a, b in ab:
                        u, v = (a, b) if k % 2 == 0 else (b, a)
                        nc.tensor.matmul(
                            P_[32 * v : 32 * v + 32, 512 * u : 512 * u + 512],
                            lhsT=Wf[32 * u : 32 * u + Kd, 32 * k : 32 * k + 32],
                            rhs=H[32 * u : 32 * u + Kd, 512 * v : 512 * v + 512],
                            start=True,
                            stop=True,
                            tile_position=(32 * u, 32 * v),
                        )
            if last:
                Hn = sp.tile([128, 2048], f32)
                # When the tail path is active Wout is pre-scaled by 0.5
                # (the DVE tail computes sigma via tanh(z/2)); the ACT
                # fallback (last pair) compensates with scale=2.
                nc.scalar.activation(
                    Hn[:], P_[:], Sigmoid, scale=2.0 if tail_on else 1.0
                )
            else:
                # Output dtype f32r iff the next layer's matmul is f32r.
                h_dt = f32r if k + 1 >= f32r_from else f32
                Hn = hp.tile([128, 2048], h_dt)
                nc.scalar.activation(Hn[:], P_[:], Tanh)
            return Hn

        def tail_layer(H, k):
            """One tail layer on PE+DVE: block-diag f32r matmul, then
            clamp -> rational tanh (NUM, DEN, 1/Q, N*Qi) on the Vector
            engine.  The PSUM slab is freed by the clamp."""
            last = k == N_LAYERS - 1
            P_ = pp.tile([128, 2048], f32)
            for a in range(4):
                nc.tensor.matmul(
                    P_[:, 512 * a : 512 * a + 512],
                    lhsT=Wbr[:, 128 * (k - 1) : 128 * k],
                    rhs=H[:, 512 * a : 512 * a + 512],
                    start=True,
                    stop=True,
                    tile_position=(0, 0),
                )
            C_ = tp.tile([128, 2048], f32)
            nc.vector.tensor_scalar(
                C_[:], P_[:], A_CLAMP, -A_CLAMP,
                mybir.AluOpType.min, mybir.AluOpType.max,
            )
            N_ = tp.tile([128, 2048], f32)
            nc.vector._custom_dve(
                OP_NUM, out=N_[:], in0=C_[:],
                s0=RAT_P[2], s1=RAT_P[1], imm2=RAT_P[0],
            )
            Q_ = tp.tile([128, 2048], f32)
            nc.vector._custom_dve(
                OP_DEN, out=Q_[:], in0=C_[:],
                s0=RAT_Q[2], s1=RAT_Q[1], imm2=RAT_Q[0],
            )
            # 1/Q written over the clamp tile (dead after DEN; the engine
            # is in-order so the WAR resolves trivially).
            Qi_ = C_
            nc.vector._custom_dve(
                OP_RECIP, out=Qi_[:], in0=Q_[:], **RECIP_CONSTS
            )
            if last:
                S_ = sp.tile([128, 2048], f32)
                nc.vector._custom_dve(
                    OP_MA, out=S_[:], in0=N_[:], in1=Qi_[:], s0=0.5, s1=0.5
                )
                return S_
            Hn = thp.tile([128, 2048], f32r)
            nc.vector._custom_dve(
                OP_MA, out=Hn[:].bitcast(f32), in0=N_[:], in1=Qi_[:],
                s0=1.0, s1=0.0,
            )
            return Hn

        if tail_on:
            OP_RECIP = dve_ops.RECIPROCAL_APPROX_FAST
            RECIP_CONSTS = dve_ops.RECIP_APPROX_FAST_CONSTS

        def store_out(s, S, eng):
            # Block-transpose back to pixel-major so the scatter uses 12B
            # chunks with a 32-row outer dim (spreads across all DMA engines):
            # SR[32a+p, 32c+f] = S[32a+f, 32c+p] = out feature f of pixel
            # s*8192 + a*2048 + 32c + p.
            SR = sp.tile([128, 2048], f32)
            nc.vector.transpose(SR[:], S[:])
            for a in range(4):
                p0 = s * ST_PIX + a * 2048
                eng.dma_start(
                    o_ap[p0 : p0 + 2048, :].rearrange("(c p) f -> p c f", c=64, p=32),
                    SR[32 * a : 32 * a + 32, :].rearrange(
                        "p (c f) -> p c f", c=64, f=32
                    )[:, :, 0:3],
                )

        def make_tail_steps(sA, sB, HA, HB):
            """Closures for the pair's tail (layers tail_from..23, streams
            A/B interleaved); executed later, spread through the NEXT
            pair's head so the DVE chains overlap ACT work."""
            st = {0: HA, 1: HB}
            steps = []
            for k in range(tail_from, N_LAYERS):
                for i, sidx in ((0, sA), (1, sB)):
                    def step(i=i, k=k, sidx=sidx):
                        Hn = tail_layer(st[i], k)
                        if k == N_LAYERS - 1:
                            store_out(sidx, Hn, nc.sync)
                        else:
                            st[i] = Hn
                    steps.append(step)
            return steps

        # Software-pipelined staging: pair p+1's loads (DMA + DVE
        # transpose) are issued before pair p's store transposes so the
        # DVE queue doesn't head-of-line block the next pair's first tanh
        # behind stores that wait on this pair's sigmoid.  With the DVE
        # tail active, pair p's tail steps are deferred and interleaved
        # into pair p+1's head (one step every 3 head layers).
        XA, XB = load_x(0, nc.sync), load_x(1, nc.sync)
        pending = []
        for pair in range(n_pairs):
            sA, sB = 2 * pair, 2 * pair + 1
            HA, HB = XA, XB
            if pair + 1 < n_pairs:
                XA, XB = load_x(sA + 2, nc.sync), load_x(sB + 2, nc.sync)
            last_pair = pair == n_pairs - 1
            head_n = N_LAYERS if (last_pair or not tail_on) else tail_from
            pi = 0
            for k in range(head_n):
                HA = layer(HA, k)
                HB = layer(HB, k)
                while pending and 3 * pi + 2 <= k:
                    pending.pop(0)()
                    pi += 1
            for s in pending:
                s()
            pending = []
            if head_n == N_LAYERS:
                store_out(sA, HA, nc.sync)
                store_out(sB, HB, nc.sync)
            else:
                pending = make_tail_steps(sA, sB, HA, HB)
        for s in pending:
            s()

    nc.compile()
    return nc


def _get_program(n_pairs, f32r_from, tail_from):
    key = (n_pairs, f32r_from, tail_from)
    if key not in _BUILD_CACHE:
        _BUILD_CACHE[key] = _build(n_pairs, f32r_from, tail_from)
    return _BUILD_CACHE[key]


def _pack_weights(W1, Whid, Wout, half_out):
    """[128, 24*32]: per partition-group u, column block l*32 holds W_l.T."""
    WT = np.zeros((N_LAYERS, 32, 32), np.float32)
    WT[0, :D_IN, :] = np.asarray(W1, np.float32).T
    WT[1:23] = np.transpose(np.asarray(Whid, np.float32), (0, 2, 1))
    WT[23, :, :3] = np.asarray(Wout, np.float32).T
    if half_out:
        # DVE tail computes sigmoid as 0.5 + 0.5*tanh(z) with z = 0.5*x;
        # fold the 0.5 into the output weights.
        WT[23] *= 0.5
    Wh = np.zeros((128, N_LAYERS * 32), np.float32)
    blocks = WT.transpose(1, 0, 2).reshape(32, N_LAYERS * 32)
    for u in range(4):
        Wh[32 * u : 32 * u + 32, :] = blocks
    Wbd = np.zeros((128, 23, 128), np.float32)
    for g in range(4):
        Wbd[32 * g : 32 * g + 32, :, 32 * g : 32 * g + 32] = WT[1:24].transpose(
            1, 0, 2
        )
    return Wh, Wbd.reshape(128, 23 * 128)


def _run(x, W1, Whid, Wout, trace=False, n_pairs=None, **spmd_kwargs):
    from concourse.bass_utils import run_bass_kernel_spmd

    if n_pairs is None:
        n_pairs = int(os.environ.get("BASSK_PAIRS", N_PAIRS))
    f32r_from = int(os.environ.get("BASSK_F32R_FROM", 12))
    tail_from = int(os.environ.get("BASSK_TAIL_FROM", 21))
    nc = _get_program(n_pairs, f32r_from, tail_from)

    x = np.ascontiguousarray(np.asarray(x, np.float32))
    assert x.shape == (N_PIX, D_IN), x.shape
    Wh, Wbd = _pack_weights(W1, Whid, Wout, half_out=tail_from < N_LAYERS)

    in_maps = [
        {"x": x[i * P_CORE : (i + 1) * P_CORE], "w": Wh, "wbd": Wbd}
        for i in range(N_CORES)
    ]
    res = run_bass_kernel_spmd(
        nc, in_maps, list(range(N_CORES)), trace=trace, **spmd_kwargs
    )
    out = np.concatenate([res.results[i]["o"] for i in range(N_CORES)], axis=0)
    return out, res


def kernel(x, W1, Whid, Wout):
    out, _ = _run(x, W1, Whid, Wout)
    return out



# revision 31
# speedup vs baseline: 7.1405x; 6.8823x over previous
"""Trainium2 Bass kernel for a CPPN-style dense MLP forward pass.

Network (per pixel): 11 -> [32 x 23 tanh layers] -> 3 sigmoid.
  h = tanh(x @ W1.T); 22x: h = tanh(h @ Whid[l].T); out = sigmoid(h @ Wout.T)

Full inputs:  x [4194304, 11] f32, W1 [32, 11], Whid [22, 32, 32], Wout [3, 32]
Full output:  [4194304, 3] f32

Strategy: pure data parallel over 8 NeuronCores (pixels split 8 ways,
weights replicated).  Per core the kernel is ScalarE(tanh)-throughput
bound (~1963ns per [128,2048] ACTIVATE, 96% busy), so the design works
to (a) keep ACT saturated and (b) move activations OFF the ACT engine
wherever the chaotic error growth allows.

Layout per core: pixels processed in "supertiles" of 16 tiles x 512
pixels = 8192 pixels, feature-major ([32 features, 512 pixels] per
32-partition group).  Each layer fills one [128, 2048] PSUM half (4
banks) then drains it with one ACT instruction; two supertile streams
interleave so the PE fills one half while ACT drains the other.

Default configuration: all 24 layers as fp32 16-way tile_position-
packed 32x32 matmuls + ACT tanh/sigmoid (measured ~3.13 ms, rel err
2e-4; ACT is 96% busy at its ~1963ns/ACTIVATE floor, so this is within
~2% of the structure's ceiling).

Two env-gated experimental tiers are kept in the code (the tanh chain
is chaotic, error amplifies ~1.39x/layer, gate is 2e-2 L2):
 - BASSK_F32R_FROM=12: layers 12+ via f32r (12-bit mantissa, single-
   pass, block-diagonal [128,128] full-array) matmuls.  5.3e-3 final
   L2, same speed (ACT-bound either way).
 - BASSK_TAIL_FROM=22: last layers' activations on the Vector engine
   via custom DVE microcode: clamp (tensor_scalar), tanh(x) =
   x*P(x^2)/Q(x^2) (two custom Horner ops, stock approx-reciprocal,
   multiply; ~9e-5 max err); sigmoid as 0.5+0.5*tanh(z/2) with the 0.5
   folded into Wout.  Tail chains of pair p are deferred into pair
   p+1's ACT-bound head.  Numerically correct but measured ~0.25 ms
   SLOWER: each chain's PSUM slab is freed by its DVE clamp, and the
   clamp's latency sits on the ACT stream's critical path through the
   2-slab PSUM rotation, stalling ACT ~2-4us per chain.

I/O avoids small-packet DMA death: x is loaded pixel-major with 44B
contiguous chunks and block-transposed to feature-major on the VectorE
32x32 STREAM_TRANSPOSE (issued a pair ahead to dodge head-of-line
blocking); the output is block-transposed back so the store scatters
12B/pixel chunks across all 16 DMA engines.  DMAs issue from SyncE.
"""

import os
import sys

if "/opt/trn_rl_repo" not in sys.path:
    sys.path.insert(0, "/opt/trn_rl_repo")

import numpy as np

N_CORES = 8
N_PIX = 4194304
P_CORE = N_PIX // N_CORES      # 524288 pixels per core
D_IN = 11
D_H = 32
N_LAYERS = 24                  # 1 input + 22 hidden + 1 output
F = 512                        # pixels per tile (one PSUM bank of fp32)
ST_PIX = 16 * F                # 8192 pixels per supertile
N_ST = P_CORE // ST_PIX        # 64 supertiles per core
N_PAIRS = N_ST // 2            # 32 interleaved supertile pairs

_BUILD_CACHE = {}

# Rational tanh for the DVE tail: tanh(x) ~= x*P(x^2)/Q(x^2) on [-5, 5]
# (clamped), P/Q cubic with unit constant terms (fit maxerr 6e-8; the
# clamp's saturation error 9.1e-5 dominates).  1/Q via the stock
# RECIPROCAL_APPROX_FAST op (~51 ULP).
A_CLAMP = 5.0
RAT_P = (0.12547021940727285, 0.0025078209274810507, 5.175999166403778e-06)
RAT_Q = (0.458802999833475, 0.02210980598237151, 0.00016889688396419473)

_DVE_OPS = None


def _register_dve_ops():
    """Register the 3 custom DVE ops (append-only; idempotent)."""
    global _DVE_OPS
    if _DVE_OPS is not None:
        return _DVE_OPS
    import numpy as np_
    import concourse.dve_ops as dve_ops
    from concourse.dve_ops import DveOp
    from concourse.dve_spec import (
        C0, C1, C2, One, Spec, Src0, Src1, lower, sq, _has_src1,
    )
    from concourse.dve_uop import DveOpSpec

    existing = {op.name: op for op in dve_ops.OPS}

    def mk(name, spec):
        if name in existing:
            return existing[name]
        op = DveOp(name, spec, subdim=False, uops_sha={})
        dve_ops.OPS.append(op)
        dve_ops._SUB_OPCODE_FOR_NAME[name] = (
            dve_ops._CUSTOM_DVE_ROW_BASE + len(dve_ops.OPS) - 1
        )
        dve_ops.CUSTOM_DVE_SPECS[name] = spec
        compiled = DveOpSpec(
            name=name,
            opcode=dve_ops.get_dve_sub_opcode(name),
            uops=lower(spec, ver="v3"),
            rd1_en=_has_src1(spec),
        )
        op.uops_sha["v3"] = compiled.sha("v3")
        return op

    U1 = sq(Src0)
    num = mk("TANH_NUM_ANT2", Spec(
        body=(((C0 * U1 + C1) * U1 + C2) * U1 + One) * Src0,
        reference=lambda in0, in1, s0, s1, imm2: (
            (((s0 * (in0 * in0) + s1) * (in0 * in0) + imm2) * (in0 * in0)
             + np_.float32(1.0)) * in0
        ).astype(np_.float32),
    ))
    U2 = sq(Src0)
    den = mk("TANH_DEN_ANT2", Spec(
        body=((C0 * U2 + C1) * U2 + C2) * U2 + One,
        reference=lambda in0, in1, s0, s1, imm2: (
            ((s0 * (in0 * in0) + s1) * (in0 * in0) + imm2) * (in0 * in0)
            + np_.float32(1.0)
        ).astype(np_.float32),
    ))
    ma = mk("MUL_ADDC_ANT2", Spec(
        body=(Src0 * Src1) * C0 + C1,
        reference=lambda in0, in1, s0, s1, imm2: (
            in0 * in1 * s0 + s1
        ).astype(np_.float32),
    ))
    _DVE_OPS = (num, den, ma)
    return _DVE_OPS


def _build(n_pairs, f32r_from, tail_from):
    """Build + bass-compile the per-core program. Returns the Bacc object."""
    import concourse.bass as bass  # noqa: F401
    import concourse.tile as tile
    from concourse import bacc, mybir
    import concourse.dve_ops as dve_ops
    from contextlib import ExitStack

    f32 = mybir.dt.float32
    f32r = mybir.dt.float32r
    Tanh = mybir.ActivationFunctionType.Tanh
    Sigmoid = mybir.ActivationFunctionType.Sigmoid
    tail_on = tail_from < N_LAYERS
    if tail_on:
        OP_NUM, OP_DEN, OP_MA = _register_dve_ops()
        assert tail_from > f32r_from, "tail layers must be in the f32r range"

    nc = bacc.Bacc(
        "TRN2", target_bir_lowering=False, debug=False, num_devices=N_CORES
    )
    x_ap = nc.dram_tensor("x", [P_CORE, D_IN], f32, kind="ExternalInput").ap()
    w_ap = nc.dram_tensor("w", [128, N_LAYERS * 32], f32, kind="ExternalInput").ap()
    wbd_ap = nc.dram_tensor("wbd", [128, 23 * 128], f32, kind="ExternalInput").ap()
    o_ap = nc.dram_tensor("o", [P_CORE, 3], f32, kind="ExternalOutput").ap()

    with tile.TileContext(nc) as tc, ExitStack() as ctx:
        wp = ctx.enter_context(tc.tile_pool(name="wp", bufs=1))
        xrp = ctx.enter_context(tc.tile_pool(name="xrp", bufs=2 if tail_on else 4))
        xp = ctx.enter_context(tc.tile_pool(name="xp", bufs=2 if tail_on else 4))
        hp = ctx.enter_context(tc.tile_pool(name="hp", bufs=3 if tail_on else 4))
        sp = ctx.enter_context(tc.tile_pool(name="sp", bufs=2 if tail_on else 4))
        if tail_on:
            tp = ctx.enter_context(tc.tile_pool(name="tp", bufs=2))
            thp = ctx.enter_context(tc.tile_pool(name="thp", bufs=2))
        pp = ctx.enter_context(tc.tile_pool(name="pp", bufs=2, space="PSUM"))

        Wf = wp.tile([128, N_LAYERS * 32], f32)
        nc.sync.dma_start(Wf[:], w_ap[:])
        if f32r_from < N_LAYERS:
            # Trailing layers run as full-array [128,128] block-diagonal f32r
            # matmuls (f32r only supports column-group 0, so no 16-way
            # packing); the explicit scalar copy is the required f32r
            # rounding producer.  Chaotic error growth caps how many layers
            # tolerate ~12-bit mantissa: f32r from layer 12 measures ~5e-3
            # final L2 (gate is 2e-2); keep f32r_from even so the fp32
            # 16-way grid swaps cancel by the store.
            Wbf = wp.tile([128, 23 * 128], f32)
            nc.sync.dma_start(Wbf[:], wbd_ap[:])
            Wbr = wp.tile([128, 23 * 128], f32r)
            nc.scalar.copy(Wbr[:], Wbf[:])

        def load_x(s, eng):
            # Stage 1: pixel-major load, 44B contiguous chunks per pixel row,
            # laid out so that 32x32 block-transpose yields feature-major
            # tiles: XR[32u+p, 32c+f] = x[s*8192 + u*2048 + 32c + p, f].
            XR = xrp.tile([128, 2048], f32)
            for u in range(4):
                p0 = s * ST_PIX + u * 2048
                eng.dma_start(
                    XR[32 * u : 32 * u + 32, :].rearrange(
                        "p (c f) -> p c f", c=64, f=32
                    )[:, :, 0:D_IN],
                    x_ap[p0 : p0 + 2048, :].rearrange("(c p) f -> p c f", c=64, p=32),
                )
            # Stage 2: DVE 32x32 block transpose -> X[32u+f, 32c+p].
            X = xp.tile([128, 2048], f32)
            nc.vector.transpose(X[:], XR[:])
            return X


        def layer(H, k):
            """One layer for one supertile: matmuls + one ACT."""
            Kd = D_IN if k == 0 else 32
            last = k == N_LAYERS - 1
            P_ = pp.tile([128, 2048], f32)
            if k >= f32r_from:
                # 4 block-diagonal full-array f32r matmuls, layout-
                # preserving: tile (a,g) stays at [32g, 512a].  Block k-1
                # of Wbr holds layer k's weights (incl. the padded output
                # layer at block 22).
                for a in range(4):
                    nc.tensor.matmul(
                        P_[:, 512 * a : 512 * a + 512],
                        lhsT=Wbr[:, 128 * (k - 1) : 128 * k],
                        rhs=H[:, 512 * a : 512 * a + 512],
                        start=True,
                        stop=True,
                        tile_position=(0, 0),
                    )
            else:
                # fp32 16-way tile-position packing; iterate so consecutive
                # matmuls land on different PE row groups (LDWEIGHTS only
                # pulls ahead of in-flight MMs when row_grp differs).
                ab = [(a, b) for b in range(4) for a in range(4)]
                if k % 2 == 1:
                    ab = [(a, b) for a in range(4) for b in range(4)]
                for a, b in ab:
                        u, v = (a, b) if k % 2 == 0 else (b, a)
                        nc.tensor.matmul(
                            P_[32 * v : 32 * v + 32, 512 * u : 512 * u + 512],
                            lhsT=Wf[32 * u : 32 * u + Kd, 32 * k : 32 * k + 32],
                            rhs=H[32 * u : 32 * u + Kd, 512 * v : 512 * v + 512],
                            start=True,
                            stop=True,
                            tile_position=(32 * u, 32 * v),
                        )
            if last:
                Hn = sp.tile([128, 2048], f32)
                # When the tail path is active Wout is pre-scaled by 0.5
                # (the DVE tail computes sigma via tanh(z/2)); the ACT
                # fallback (last pair) compensates with scale=2.
                nc.scalar.activation(
                    Hn[:], P_[:], Sigmoid, scale=2.0 if tail_on else 1.0
                )
            else:
                # Output dtype f32r iff the next layer's matmul is f32r.
                h_dt = f32r if k + 1 >= f32r_from else f32
                Hn = hp.tile([128, 2048], h_dt)
                nc.scalar.activation(Hn[:], P_[:], Tanh)
            return Hn

        def tail_layer(H, k):
            """One tail layer on PE+DVE: block-diag f32r matmul, then
            clamp -> rational tanh (NUM, DEN, 1/Q, N*Qi) on the Vector
            engine.  The PSUM slab is freed by the clamp."""
            last = k == N_LAYERS - 1
            P_ = pp.tile([128, 2048], f32)
            for a in range(4):
                nc.tensor.matmul(
                    P_[:, 512 * a : 512 * a + 512],
                    lhsT=Wbr[:, 128 * (k - 1) : 128 * k],
                    rhs=H[:, 512 * a : 512 * a + 512],
                    start=True,
                    stop=True,
                    tile_position=(0, 0),
                )
            C_ = tp.tile([128, 2048], f32)
            nc.vector.tensor_scalar(
                C_[:], P_[:], A_CLAMP, -A_CLAMP,
                mybir.AluOpType.min, mybir.AluOpType.max,
            )
            N_ = tp.tile([128, 2048], f32)
            nc.vector._custom_dve(
                OP_NUM, out=N_[:], in0=C_[:],
                s0=RAT_P[2], s1=RAT_P[1], imm2=RAT_P[0],
            )
            Q_ = tp.tile([128, 2048], f32)
            nc.vector._custom_dve(
                OP_DEN, out=Q_[:], in0=C_[:],
                s0=RAT_Q[2], s1=RAT_Q[1], imm2=RAT_Q[0],
            )
            # 1/Q written over the clamp tile (dead after DEN; the engine
            # is in-order so the WAR resolves trivially).
            Qi_ = C_
            nc.vector._custom_dve(
                OP_RECIP, out=Qi_[:], in0=Q_[:], **RECIP_CONSTS
            )
            if last:
                S_ = sp.tile([128, 2048], f32)
                nc.vector._custom_dve(
                    OP_MA, out=S_[:], in0=N_[:], in1=Qi_[:], s0=0.5, s1=0.5
                )
                return S_
            Hn = thp.tile([128, 2048], f32r)
            nc.vector._custom_dve(
                OP_MA, out=Hn[:], in0=N_[:], in1=Qi_[:], s0=1.0, s1=0.0
            )
            return Hn

        if tail_on:
            OP_RECIP = dve_ops.RECIPROCAL_APPROX_FAST
            RECIP_CONSTS = dve_ops.RECIP_APPROX_FAST_CONSTS

        def store_out(s, S, eng):
            # Block-transpose back to pixel-major so the scatter uses 12B
            # chunks with a 32-row outer dim (spreads across all DMA engines):
            # SR[32a+p, 32c+f] = S[32a+f, 32c+p] = out feature f of pixel
            # s*8192 + a*2048 + 32c + p.
            SR = sp.tile([128, 2048], f32)
            nc.vector.transpose(SR[:], S[:])
            for a in range(4):
                p0 = s * ST_PIX + a * 2048
                eng.dma_start(
                    o_ap[p0 : p0 + 2048, :].rearrange("(c p) f -> p c f", c=64, p=32),
                    SR[32 * a : 32 * a + 32, :].rearrange(
                        "p (c f) -> p c f", c=64, f=32
                    )[:, :, 0:3],
                )

        def make_tail_steps(sA, sB, HA, HB):
            """(due_k, closure) steps for the pair's tail (layers
            tail_from..23, streams A/B alternating) plus the two store
            transposes; executed later, spread through the NEXT pair's
            head at fixed layer slots so each DVE chain gets its own
            ~12us window (per-stream chain spacing is ~24us, so chain
            latency never cascades) and the clamp that frees the PSUM
            slab is never queued behind other DVE work."""
            st = {0: HA, 1: HB}
            n_steps = 2 * (N_LAYERS - tail_from) + 2
            slots = [
                round(2 + 18 * j / (n_steps - 1)) for j in range(n_steps)
            ]
            steps = []
            j = 0
            for k in range(tail_from, N_LAYERS):
                for i in (0, 1):
                    def step(i=i, k=k):
                        st[i] = tail_layer(st[i], k)
                    steps.append((slots[j], step))
                    j += 1
            for i, sidx in ((0, sA), (1, sB)):
                def sstep(i=i, sidx=sidx):
                    store_out(sidx, st[i], nc.sync)
                steps.append((slots[j], sstep))
                j += 1
            return steps

        # Software-pipelined staging: pair p+1's loads (DMA + DVE
        # transpose) are issued before pair p's store transposes so the
        # DVE queue doesn't head-of-line block the next pair's first tanh
        # behind stores that wait on this pair's sigmoid.  With the DVE
        # tail active, pair p's tail steps are deferred and interleaved
        # into pair p+1's head (one step every 3 head layers).
        XA, XB = load_x(0, nc.sync), load_x(1, nc.sync)
        pending = []
        for pair in range(n_pairs):
            sA, sB = 2 * pair, 2 * pair + 1
            HA, HB = XA, XB
            if pair + 1 < n_pairs:
                XA, XB = load_x(sA + 2, nc.sync), load_x(sB + 2, nc.sync)
            last_pair = pair == n_pairs - 1
            head_n = N_LAYERS if (last_pair or not tail_on) else tail_from
            for k in range(head_n):
                HA = layer(HA, k)
                HB = layer(HB, k)
                while pending and pending[0][0] <= k:
                    pending.pop(0)[1]()
            for _, s in pending:
                s()
            pending = []
            if head_n == N_LAYERS:
                # Both stores issue from SyncE.  (Tried: splitting the
                # store issues across GPSIMD+SyncE to parallelize the
                # 2-8us/sub-DMA descriptor generation that serializes the
                # epilogue -- the Pool engine's DMA-issue path measured
                # ~700us SLOWER overall; SyncE it is.)
                store_out(sA, HA, nc.sync)
                store_out(sB, HB, nc.sync)
            else:
                pending = make_tail_steps(sA, sB, HA, HB)
        for _, s in pending:
            s()

    nc.compile()
    return nc


def _get_program(n_pairs, f32r_from, tail_from):
    key = (n_pairs, f32r_from, tail_from)
    if key not in _BUILD_CACHE:
        _BUILD_CACHE[key] = _build(n_pairs, f32r_from, tail_from)
    return _BUILD_CACHE[key]


def _pack_weights(W1, Whid, Wout, half_out):
    """[128, 24*32]: per partition-group u, column block l*32 holds W_l.T."""
    WT = np.zeros((N_LAYERS, 32, 32), np.float32)
    WT[0, :D_IN, :] = np.asarray(W1, np.float32).T
    WT[1:23] = np.transpose(np.asarray(Whid, np.float32), (0, 2, 1))
    WT[23, :, :3] = np.asarray(Wout, np.float32).T
    if half_out:
        # DVE tail computes sigmoid as 0.5 + 0.5*tanh(z) with z = 0.5*x;
        # fold the 0.5 into the output weights.
        WT[23] *= 0.5
    Wh = np.zeros((128, N_LAYERS * 32), np.float32)
    blocks = WT.transpose(1, 0, 2).reshape(32, N_LAYERS * 32)
    for u in range(4):
        Wh[32 * u : 32 * u + 32, :] = blocks
    Wbd = np.zeros((128, 23, 128), np.float32)
    for g in range(4):
        Wbd[32 * g : 32 * g + 32, :, 32 * g : 32 * g + 32] = WT[1:24].transpose(
            1, 0, 2
        )
    return Wh, Wbd.reshape(128, 23 * 128)


def _run(x, W1, Whid, Wout, trace=False, n_pairs=None, **spmd_kwargs):
    from concourse.bass_utils import run_bass_kernel_spmd

    if n_pairs is None:
        n_pairs = int(os.environ.get("BASSK_PAIRS", N_PAIRS))
    # Defaults ship the measured-fastest config: plain fp32 16-way with
    # the prefetch-reordered pipeline (~3.13 ms, rel err 2e-4).  The f32r
    # block-diag path (BASSK_F32R_FROM=12: same speed, rel err 5.3e-3)
    # and the custom-DVE rational-tanh tail (BASSK_TAIL_FROM=22: measured
    # ~0.25 ms SLOWER -- the DVE chains stall the ACT stream through the
    # 2-slab PSUM rotation) are kept behind env flags for reference.
    f32r_from = int(os.environ.get("BASSK_F32R_FROM", 24))
    tail_from = int(os.environ.get("BASSK_TAIL_FROM", 24))
    nc = _get_program(n_pairs, f32r_from, tail_from)

    x = np.ascontiguousarray(np.asarray(x, np.float32))
    assert x.shape == (N_PIX, D_IN), x.shape
    Wh, Wbd = _pack_weights(W1, Whid, Wout, half_out=tail_from < N_LAYERS)

    in_maps = [
        {"x": x[i * P_CORE : (i + 1) * P_CORE], "w": Wh, "wbd": Wbd}
        for i in range(N_CORES)
    ]
    res = run_bass_kernel_spmd(
        nc, in_maps, list(range(N_CORES)), trace=trace, **spmd_kwargs
    )
    out = np.concatenate([res.results[i]["o"] for i in range(N_CORES)], axis=0)
    return out, res


def kernel(x, W1, Whid, Wout):
    out, _ = _run(x, W1, Whid, Wout)
    return out

